# revision 42
# baseline (speedup 1.0000x reference)
"""Trainium2 Bass kernel for a talking-heads MHSA block.

Reference computation (B=4, P=2048, D=512, H=8, DF=64, fp32):
    q = (x @ Wq) / sqrt(DF);  k = x @ Wk;  v = x @ Wv      (per-head reshape)
    attn[b,h]   = q_h k_h^T
    attn2[b,g]  = sum_h Wtalk[g,h] attn[b,h]               (talking heads)
    P           = softmax(attn2 + bias, axis=-1)
    out         = concat_g(P_g v_g) @ Wo

Sharding: 8 cores, data-parallel: core c -> batch b=c//2, query-half s=c%2
(1024 query rows, all heads, full 2048 keys). No collectives.

Per-core algorithm (bf16 matmuls, fp32 logits, zero on-chip transposes):
  - host pre-transposes x -> x^T and the bias slice -> bias^T[g, q, p]
  - talking-heads mix is folded into QK: S_mixed[g] = (Wtalk[g,h]/8 * Q)
    contracted over all 512 features against K -> one dense 512-deep matmul
  - S^T[q, p] accumulates in PSUM fp32; DVE adds fp32 bias^T in place; ACT
    computes exp -> bf16 probabilities (no max-subtraction: logits are
    bounded ~+-7, mathematically identical)
  - AV uses exp(S^T) directly as the bf16 moving operand; a ones-column in
    V' produces the softmax denominators in PSUM partition 64 for free
  - normalization is applied after AV (linear), then the output projection
    consumes out^T as lhsT directly.
"""
import sys
from contextlib import ExitStack

import numpy as np

if "/opt/trn_rl_repo" not in sys.path:
    sys.path.insert(0, "/opt/trn_rl_repo")

B, P, D = 4, 2048, 512
H, DF = 8, 64
G = H                 # output head groups
PH = P // 2           # query rows per core
DC = D // 128         # 4 contraction chunks for d
EC = (H * DF) // 128  # 4 chunks for e = (h, df)
QC = P // 128         # 16 key chunks
VW = DF + 1           # V' width per group: 64 cols of V + ones column
N_CORES = 8

_CACHE = {}
LAST_RESULTS = None


def _build_program():
    import concourse.mybir as mybir
    import concourse.tile as tile
    from concourse import bacc

    f32 = mybir.dt.float32
    bf16 = mybir.dt.bfloat16
    ACT = mybir.ActivationFunctionType

    nc = bacc.Bacc("TRN2", target_bir_lowering=False, debug=False)
    # x and the projection weights arrive pre-cast to bf16: halves the HBM
    # read traffic during the DMA-bound staging phase
    xt = nc.dram_tensor("xt", [D, P], bf16, kind="ExternalInput").ap()
    xqt = nc.dram_tensor("xqt", [D, PH], bf16, kind="ExternalInput").ap()
    biast = nc.dram_tensor("biast", [G, P, PH], bf16, kind="ExternalInput").ap()
    wq = nc.dram_tensor("wq", [D, H * DF], bf16, kind="ExternalInput").ap()
    wk = nc.dram_tensor("wk", [D, H * DF], bf16, kind="ExternalInput").ap()
    wv = nc.dram_tensor("wv", [D, H * DF], bf16, kind="ExternalInput").ap()
    wo = nc.dram_tensor("wo", [H * DF, D], bf16, kind="ExternalInput").ap()
    wt = nc.dram_tensor("wt", [H * DF, G], f32, kind="ExternalInput").ap()
    y = nc.dram_tensor("y", [PH, D], f32, kind="ExternalOutput").ap()

    with tile.TileContext(nc) as tc, ExitStack() as ctx:
        persist = ctx.enter_context(tc.tile_pool(name="persist", bufs=1))
        qt_sb = persist.tile([128, EC * PH], bf16, tag="qt")      # Q^T [e, p]
        kt_sb = persist.tile([128, EC * P], bf16, tag="kt")       # K^T [e, q]
        v_sb = persist.tile([128, QC * G * VW], bf16, tag="v")    # V' [q, g*65+c]
        wo_sb = persist.tile([128, EC * D], bf16, tag="wo")
        wt_sb = persist.tile([128, EC * G], f32, tag="wt")
        ocat_sb = persist.tile([128, EC * PH], bf16, tag="ocat")  # out^T [e, p]

        def cast_load(dst_tile, dram_ap, n):
            # SWDGE cast f32 DRAM -> bf16 SBUF, one DMA per tensor
            nc.gpsimd.dma_start(
                dst_tile[:].rearrange("p (c m) -> p c m", c=n),
                dram_ap.rearrange("(c p) m -> p c m", p=128))

        # staging tiles stay live into phase C (the V projection runs there)
        stage = ctx.enter_context(tc.tile_pool(name="stage", bufs=1))
        xt_sb = stage.tile([128, DC * P], bf16, tag="xt")
        xqt_sb = stage.tile([128, DC * PH], bf16, tag="xqt")
        wq_sb = stage.tile([128, DC * D], bf16, tag="wq")
        wk_sb = stage.tile([128, DC * D], bf16, tag="wk")
        wv_sb = stage.tile([128, DC * D], bf16, tag="wv")

        # ---------- phase B: staging + Q/K projections ----------
        with ExitStack() as pb:
            # bf16 inputs, few descriptors; xqt split in pc-halves so the Q
            # projection's first half starts after ~1MB instead of the lot
            xqt_v = xqt.rearrange("(c p) m -> p c m", p=128)
            xqt_sbv = xqt_sb[:].rearrange("p (c m) -> p c m", c=DC)
            nc.gpsimd.dma_start(xqt_sbv[:, :, 0:512], xqt_v[:, :, 0:512])
            cast_load(wq_sb, wq, DC)
            nc.gpsimd.dma_start(xqt_sbv[:, :, 512:1024], xqt_v[:, :, 512:1024])
            cast_load(wk_sb, wk, DC)
            # xt in qn-chunks: the qn-outer K projection streams as they land
            xt_v = xt.rearrange("(c p) m -> p c m", p=128)
            xt_sbv = xt_sb[:].rearrange("p (c m) -> p c m", c=DC)
            for qn in range(P // 512):
                nc.gpsimd.dma_start(
                    xt_sbv[:, :, qn * 512:(qn + 1) * 512],
                    xt_v[:, :, qn * 512:(qn + 1) * 512])
            cast_load(wv_sb, wv, DC)
            cast_load(wo_sb, wo, EC)
            nc.sync.dma_start(
                wt_sb[:].rearrange("p (c m) -> p c m", c=EC),
                wt.rearrange("(c p) m -> p c m", p=128))

            nc.gpsimd.memset(v_sb[:], 1.0)  # ones columns of V'

            # keep phase B's PSUM footprint at 4 banks so phase C's s_pool can
            # allocate its 4 banks without waiting for all of B's PSUM users
            psA = pb.enter_context(tc.tile_pool(name="psA", bufs=2, space="PSUM"))
            psB = pb.enter_context(tc.tile_pool(name="psB", bufs=2, space="PSUM"))

            # Q^T[e, p] = Wq^T x^T (query half only); pc-outer so the first
            # xqt half feeds compute while the second is still in flight
            for pc in range(PH // 512):
                for ec in range(EC):
                    q_ps = psA.tile([128, 512], f32, tag="qps")
                    for dc in range(DC):
                        nc.tensor.matmul(
                            q_ps[:],
                            lhsT=wq_sb[:, dc * D + ec * 128: dc * D + (ec + 1) * 128],
                            rhs=xqt_sb[:, dc * PH + pc * 512: dc * PH + (pc + 1) * 512],
                            start=(dc == 0), stop=(dc == DC - 1))
                    nc.scalar.activation(
                        qt_sb[:, ec * PH + pc * 512: ec * PH + (pc + 1) * 512],
                        q_ps[:], ACT.Copy)
            # K^T[e, q] over all keys
            for qn in range(P // 512):
                for ec in range(EC):
                    k_ps = psB.tile([128, 512], f32, tag="kvps")
                    for dc in range(DC):
                        nc.tensor.matmul(
                            k_ps[:],
                            lhsT=wk_sb[:, dc * D + ec * 128: dc * D + (ec + 1) * 128],
                            rhs=xt_sb[:, dc * P + qn * 512: dc * P + (qn + 1) * 512],
                            start=(dc == 0), stop=(dc == DC - 1))
                    nc.scalar.activation(
                        kt_sb[:, ec * P + qn * 512: ec * P + (qn + 1) * 512],
                        k_ps[:], ACT.Copy)

        # ---------- phase C: attention main loop ----------
        with ExitStack() as pcs:
            qg_pool = pcs.enter_context(tc.tile_pool(name="qg", bufs=2))
            bias_pool = pcs.enter_context(tc.tile_pool(name="bias", bufs=8))
            exp_pool = pcs.enter_context(tc.tile_pool(name="exp", bufs=11))
            nrm_pool = pcs.enter_context(tc.tile_pool(name="nrm", bufs=2))
            s_pool = pcs.enter_context(tc.tile_pool(name="sps", bufs=4, space="PSUM"))
            o_pool = pcs.enter_context(tc.tile_pool(name="ops", bufs=2, space="PSUM"))

            def make_qg(g):
                # Qg^T = Q^T * (Wtalk[g, h] / sqrt(DF)) -- folds the head mix
                qg_sb = qg_pool.tile([128, EC * PH], bf16, tag="qg")
                for ec in range(EC):
                    nc.vector.tensor_scalar_mul(
                        qg_sb[:, ec * PH:(ec + 1) * PH],
                        qt_sb[:, ec * PH:(ec + 1) * PH],
                        wt_sb[:, ec * G + g: ec * G + g + 1])
                return qg_sb

            def normalize(g, o_ps):
                # normalize straight out of PSUM: out^T[df, p] / sums[p]; the
                # ones-row of V' left the denominators in o_ps row DF (the ACT
                # copy also remaps partition DF -> 0 for the reciprocal)
                sum_sb = nrm_pool.tile([1, PH], f32, tag="sum")
                nc.scalar.activation(sum_sb[:], o_ps[DF:DF + 1, :], ACT.Copy)
                r_sb = nrm_pool.tile([1, PH], f32, tag="r")
                nc.vector.reciprocal_approx_fast(r_sb[:], sum_sb[:])
                rb_sb = nrm_pool.tile([DF, PH], f32, tag="rb")
                nc.gpsimd.partition_broadcast(rb_sb[:], r_sb[:])
                po, fo = (g % 2) * DF, (g // 2) * PH
                nc.vector.tensor_mul(
                    ocat_sb[po:po + DF, fo:fo + PH], o_ps[0:DF, :], rb_sb[:])

            # flat software pipeline over all (g, qc) tiles: AV lags QK by LAG
            # tiles ACROSS head boundaries, so the PE stream is uniform and
            # never waits on the exp->bias-multiply chain or head bookkeeping.
            # The V projection is interleaved into the first 8 tiles (V' is
            # first consumed by AV at t=LAG), which keeps it off the serial
            # phase-B critical path.
            NT = G * QC
            LAG = 8
            qg_tiles = {0: make_qg(0)}
            o_pss = {}
            e_all = [None] * NT
            for t in range(NT + LAG):
                if t < NT:
                    g, qc = divmod(t, QC)
                    qg_sb = qg_tiles[g]
                    # b_sb holds exp(bias)^T; softmax as exp(s)*exp(b).
                    # head 0's tiles ride the gpsimd queue so they line up
                    # BEHIND the phase-B loads instead of stealing HBM
                    # bandwidth from them on the concurrent sync queue
                    b_sb = bias_pool.tile([128, PH], bf16, tag="bias")
                    dma_eng = nc.gpsimd if g == 0 else nc.sync
                    dma_eng.dma_start(b_sb[:], biast[g, qc * 128:(qc + 1) * 128, :])
                    e_sb = exp_pool.tile([128, PH], bf16, tag="exp")
                    e_all[t] = e_sb
                    s_pss = []
                    for pc in range(PH // 512):
                        s_ps = s_pool.tile([128, 512], f32, tag="sps")
                        s_pss.append(s_ps)
                        for ec in range(EC):
                            nc.tensor.matmul(
                                s_ps[:],
                                lhsT=kt_sb[:, ec * P + qc * 128: ec * P + (qc + 1) * 128],
                                rhs=qg_sb[:, ec * PH + pc * 512: ec * PH + (pc + 1) * 512],
                                start=(ec == 0), stop=(ec == EC - 1))
                if t < LAG:
                    # V[q, e] natural layout, scattered into V' (2 chunks/tile)
                    for qc_v in (2 * t, 2 * t + 1):
                        v_ps = s_pool.tile([128, 512], f32, tag="sps")
                        for dc in range(DC):
                            nc.tensor.matmul(
                                v_ps[:],
                                lhsT=xt_sb[:, dc * P + qc_v * 128: dc * P + (qc_v + 1) * 128],
                                rhs=wv_sb[:, dc * D:(dc + 1) * D],
                                start=(dc == 0), stop=(dc == DC - 1))
                        dst = v_sb[:, qc_v * G * VW:(qc_v + 1) * G * VW]
                        dst = dst.rearrange("p (g c) -> p g c", c=VW)[:, :, 0:DF]
                        src = v_ps[:].rearrange("p (g c) -> p g c", c=DF)
                        nc.vector.tensor_copy(dst, src)
                if t >= LAG:
                    pg, pqc = divmod(t - LAG, QC)
                    if pqc == 0:
                        op_t = o_pool.tile([VW, PH], f32, tag="ops")
                        o_pss[pg] = op_t
                    for pc in range(PH // 512):
                        nc.tensor.matmul(
                            o_pss[pg][:, pc * 512:(pc + 1) * 512],
                            lhsT=v_sb[:, pqc * G * VW + pg * VW:
                                      pqc * G * VW + (pg + 1) * VW],
                            rhs=e_all[t - LAG][:, pc * 512:(pc + 1) * 512],
                            start=(pqc == 0), stop=(pqc == QC - 1))
                    if pqc == QC - 1:
                        normalize(pg, o_pss.pop(pg))
                if t < NT:
                    for pc in range(PH // 512):
                        nc.scalar.activation(
                            e_sb[:, pc * 512:(pc + 1) * 512], s_pss[pc][:], ACT.Exp)
                        nc.vector.tensor_mul(
                            e_sb[:, pc * 512:(pc + 1) * 512],
                            e_sb[:, pc * 512:(pc + 1) * 512],
                            b_sb[:, pc * 512:(pc + 1) * 512])
                    if qc == 2 and g + 1 < G:
                        # prefetch next head's Qg while DVE is lightly loaded
                        qg_tiles[g + 1] = make_qg(g + 1)

        # ---------- phase D: output projection ----------
        # 4-group waves: each wave runs its ec 0..2 matmuls (heads 0..5, ready
        # early) while the last head's normalize chain finishes; only the ec=3
        # matmuls wait on it. bufs=4 keeps the tiles on the freed s_pool banks,
        # clear of the still-live o_ps banks.
        with ExitStack() as pd:
            y_pool = pd.enter_context(tc.tile_pool(name="yps", bufs=8, space="PSUM"))
            ysb_pool = pd.enter_context(tc.tile_pool(name="ysb", bufs=8))
            y_pss = []
            for pc in range(PH // 128):
                y_ps = y_pool.tile([128, D], f32, tag="yps")
                y_pss.append(y_ps)
                for ec in range(EC - 1):
                    nc.tensor.matmul(
                        y_ps[:],
                        lhsT=ocat_sb[:, ec * PH + pc * 128: ec * PH + (pc + 1) * 128],
                        rhs=wo_sb[:, ec * D:(ec + 1) * D],
                        start=(ec == 0), stop=False)
            for pc in range(PH // 128):
                ec = EC - 1
                nc.tensor.matmul(
                    y_pss[pc][:],
                    lhsT=ocat_sb[:, ec * PH + pc * 128: ec * PH + (pc + 1) * 128],
                    rhs=wo_sb[:, ec * D:(ec + 1) * D],
                    start=False, stop=True)
                y_sb = ysb_pool.tile([128, D], f32, tag="ysb")
                nc.scalar.activation(y_sb[:], y_pss[pc][:], ACT.Copy)
                nc.sync.dma_start(y[pc * 128:(pc + 1) * 128, :], y_sb[:])

    nc.compile()
    return nc


def kernel(x, attn_bias, Wq, Wk, Wv, Wtalk, Wo, **trace_kwargs):
    global LAST_RESULTS
    from concourse.bass_utils import run_bass_kernel_spmd

    x = np.asarray(x, dtype=np.float32)
    attn_bias = np.asarray(attn_bias, dtype=np.float32)
    Wq = np.asarray(Wq, dtype=np.float32)
    Wk = np.asarray(Wk, dtype=np.float32)
    Wv = np.asarray(Wv, dtype=np.float32)
    Wtalk = np.asarray(Wtalk, dtype=np.float32)
    Wo = np.asarray(Wo, dtype=np.float32)

    if "nc" not in _CACHE:
        _CACHE["nc"] = _build_program()
    nc = _CACHE["nc"]

    # host-side layout prep (cheap, reused across cores)
    import ml_dtypes
    bfh = ml_dtypes.bfloat16
    xts = [np.ascontiguousarray(x[b].T).astype(bfh) for b in range(B)]     # [D, P]
    xqts = [[np.ascontiguousarray(x[b, s * PH:(s + 1) * PH, :].T).astype(bfh)
             for s in range(2)] for b in range(B)]                         # [D, PH]
    # ship exp(bias)^T so the kernel can apply bias multiplicatively after exp
    biasts = [np.ascontiguousarray(
        np.exp(attn_bias[0, :, s * PH:(s + 1) * PH, :]).transpose(0, 2, 1))
        .astype(ml_dtypes.bfloat16) for s in range(2)]
    wt = np.ascontiguousarray((np.repeat(Wtalk, DF, axis=1) / np.sqrt(DF)).T
                              .astype(np.float32))                         # [512, 8]

    wq16, wk16 = Wq.astype(bfh), Wk.astype(bfh)
    wv16, wo16 = Wv.astype(bfh), Wo.astype(bfh)
    in_maps = []
    for c in range(N_CORES):
        b, s = c // 2, c % 2
        in_maps.append({
            "xt": xts[b], "xqt": xqts[b][s], "biast": biasts[s],
            "wq": wq16, "wk": wk16, "wv": wv16, "wo": wo16, "wt": wt,
        })

    res = run_bass_kernel_spmd(nc, in_maps, list(range(N_CORES)), **trace_kwargs)
    LAST_RESULTS = res

    out = np.empty((B, P, D), dtype=np.float32)
    for c in range(N_CORES):
        b, s = c // 2, c % 2
        out[b, s * PH:(s + 1) * PH, :] = res.results[c]["y"]
    return out



# revision 44
# speedup vs baseline: 1.0177x; 1.0177x over previous
"""Trainium2 Bass kernel for a talking-heads MHSA block.

Reference computation (B=4, P=2048, D=512, H=8, DF=64, fp32):
    q = (x @ Wq) / sqrt(DF);  k = x @ Wk;  v = x @ Wv      (per-head reshape)
    attn[b,h]   = q_h k_h^T
    attn2[b,g]  = sum_h Wtalk[g,h] attn[b,h]               (talking heads)
    P           = softmax(attn2 + bias, axis=-1)
    out         = concat_g(P_g v_g) @ Wo

Sharding: 8 cores, data-parallel: core c -> batch b=c//2, query-half s=c%2
(1024 query rows, all heads, full 2048 keys). No collectives.

Per-core algorithm (bf16 matmuls, fp32 logits, zero on-chip transposes):
  - host pre-transposes x -> x^T and the bias slice -> bias^T[g, q, p]
  - talking-heads mix is folded into QK: S_mixed[g] = (Wtalk[g,h]/8 * Q)
    contracted over all 512 features against K -> one dense 512-deep matmul
  - S^T[q, p] accumulates in PSUM fp32; DVE adds fp32 bias^T in place; ACT
    computes exp -> bf16 probabilities (no max-subtraction: logits are
    bounded ~+-7, mathematically identical)
  - AV uses exp(S^T) directly as the bf16 moving operand; a ones-column in
    V' produces the softmax denominators in PSUM partition 64 for free
  - normalization is applied after AV (linear), then the output projection
    consumes out^T as lhsT directly.
"""
import sys
from contextlib import ExitStack

import numpy as np

if "/opt/trn_rl_repo" not in sys.path:
    sys.path.insert(0, "/opt/trn_rl_repo")

B, P, D = 4, 2048, 512
H, DF = 8, 64
G = H                 # output head groups
PH = P // 2           # query rows per core
DC = D // 128         # 4 contraction chunks for d
EC = (H * DF) // 128  # 4 chunks for e = (h, df)
QC = P // 128         # 16 key chunks
VW = DF + 1           # V' width per group: 64 cols of V + ones column
N_CORES = 8

_CACHE = {}
LAST_RESULTS = None


def _build_program():
    import concourse.mybir as mybir
    import concourse.tile as tile
    from concourse import bacc

    f32 = mybir.dt.float32
    bf16 = mybir.dt.bfloat16
    ACT = mybir.ActivationFunctionType

    nc = bacc.Bacc("TRN2", target_bir_lowering=False, debug=False)
    # x and the projection weights arrive pre-cast to bf16: halves the HBM
    # read traffic during the DMA-bound staging phase
    xt = nc.dram_tensor("xt", [D, P], bf16, kind="ExternalInput").ap()
    xqt = nc.dram_tensor("xqt", [D, PH], bf16, kind="ExternalInput").ap()
    biast = nc.dram_tensor("biast", [G, P, PH], bf16, kind="ExternalInput").ap()
    wq = nc.dram_tensor("wq", [D, H * DF], bf16, kind="ExternalInput").ap()
    wk = nc.dram_tensor("wk", [D, H * DF], bf16, kind="ExternalInput").ap()
    wv = nc.dram_tensor("wv", [D, H * DF], bf16, kind="ExternalInput").ap()
    wo = nc.dram_tensor("wo", [H * DF, D], bf16, kind="ExternalInput").ap()
    wt = nc.dram_tensor("wt", [H * DF, G], f32, kind="ExternalInput").ap()
    y = nc.dram_tensor("y", [PH, D], f32, kind="ExternalOutput").ap()

    with tile.TileContext(nc) as tc, ExitStack() as ctx:
        persist = ctx.enter_context(tc.tile_pool(name="persist", bufs=1))
        qt_sb = persist.tile([128, EC * PH], bf16, tag="qt")      # Q^T [e, p]
        kt_sb = persist.tile([128, EC * P], bf16, tag="kt")       # K^T [e, q]
        v_sb = persist.tile([128, QC * G * VW], bf16, tag="v")    # V' [q, g*65+c]
        wo_sb = persist.tile([128, EC * D], bf16, tag="wo")
        wt_sb = persist.tile([128, EC * G], f32, tag="wt")
        ocat_sb = persist.tile([128, EC * PH], bf16, tag="ocat")  # out^T [e, p]

        def cast_load(dst_tile, dram_ap, n):
            # SWDGE cast f32 DRAM -> bf16 SBUF, one DMA per tensor
            nc.gpsimd.dma_start(
                dst_tile[:].rearrange("p (c m) -> p c m", c=n),
                dram_ap.rearrange("(c p) m -> p c m", p=128))

        # staging tiles stay live into phase C (the V projection runs there)
        stage = ctx.enter_context(tc.tile_pool(name="stage", bufs=1))
        xt_sb = stage.tile([128, DC * P], bf16, tag="xt")
        xqt_sb = stage.tile([128, DC * PH], bf16, tag="xqt")
        wq_sb = stage.tile([128, DC * D], bf16, tag="wq")
        wk_sb = stage.tile([128, DC * D], bf16, tag="wk")
        wv_sb = stage.tile([128, DC * D], bf16, tag="wv")

        # ---------- phase B: staging + Q/K projections ----------
        with ExitStack() as pb:
            # DMA order matches the interleaved K/Q emission below: each
            # 512-wide projection block's inputs land just before the PE
            # reaches it, so the PE streams through phase B with one short
            # initial wait instead of idling on whole tensors
            xt_v = xt.rearrange("(c p) m -> p c m", p=128)
            xt_sbv = xt_sb[:].rearrange("p (c m) -> p c m", c=DC)
            xqt_v = xqt.rearrange("(c p) m -> p c m", p=128)
            xqt_sbv = xqt_sb[:].rearrange("p (c m) -> p c m", c=DC)
            cast_load(wk_sb, wk, DC)
            nc.gpsimd.dma_start(xt_sbv[:, :, 0:512], xt_v[:, :, 0:512])
            nc.gpsimd.dma_start(xqt_sbv[:, :, 0:512], xqt_v[:, :, 0:512])
            cast_load(wq_sb, wq, DC)
            nc.gpsimd.dma_start(xt_sbv[:, :, 512:1024], xt_v[:, :, 512:1024])
            nc.gpsimd.dma_start(xqt_sbv[:, :, 512:1024], xqt_v[:, :, 512:1024])
            nc.gpsimd.dma_start(xt_sbv[:, :, 1024:1536], xt_v[:, :, 1024:1536])
            nc.gpsimd.dma_start(xt_sbv[:, :, 1536:2048], xt_v[:, :, 1536:2048])
            cast_load(wv_sb, wv, DC)
            cast_load(wo_sb, wo, EC)
            nc.sync.dma_start(
                wt_sb[:].rearrange("p (c m) -> p c m", c=EC),
                wt.rearrange("(c p) m -> p c m", p=128))

            nc.gpsimd.memset(v_sb[:], 1.0)  # ones columns of V'

            # keep phase B's PSUM footprint at 4 banks so phase C's s_pool can
            # allocate its 4 banks without waiting for all of B's PSUM users
            psA = pb.enter_context(tc.tile_pool(name="psA", bufs=2, space="PSUM"))
            psB = pb.enter_context(tc.tile_pool(name="psB", bufs=2, space="PSUM"))

            def q_block(pc):
                # Q^T[e, p-block] = Wq^T x^T (query half only)
                for ec in range(EC):
                    q_ps = psA.tile([128, 512], f32, tag="qps")
                    for dc in range(DC):
                        nc.tensor.matmul(
                            q_ps[:],
                            lhsT=wq_sb[:, dc * D + ec * 128: dc * D + (ec + 1) * 128],
                            rhs=xqt_sb[:, dc * PH + pc * 512: dc * PH + (pc + 1) * 512],
                            start=(dc == 0), stop=(dc == DC - 1))
                    nc.scalar.activation(
                        qt_sb[:, ec * PH + pc * 512: ec * PH + (pc + 1) * 512],
                        q_ps[:], ACT.Copy)

            def k_block(qn):
                # K^T[e, q-block] over 512 keys
                for ec in range(EC):
                    k_ps = psB.tile([128, 512], f32, tag="kvps")
                    for dc in range(DC):
                        nc.tensor.matmul(
                            k_ps[:],
                            lhsT=wk_sb[:, dc * D + ec * 128: dc * D + (ec + 1) * 128],
                            rhs=xt_sb[:, dc * P + qn * 512: dc * P + (qn + 1) * 512],
                            start=(dc == 0), stop=(dc == DC - 1))
                    nc.scalar.activation(
                        kt_sb[:, ec * P + qn * 512: ec * P + (qn + 1) * 512],
                        k_ps[:], ACT.Copy)

            k_block(0)
            q_block(0)
            k_block(1)
            q_block(1)
            k_block(2)
            k_block(3)

        # ---------- phase C: attention main loop ----------
        with ExitStack() as pcs:
            qg_pool = pcs.enter_context(tc.tile_pool(name="qg", bufs=2))
            bias_pool = pcs.enter_context(tc.tile_pool(name="bias", bufs=8))
            exp_pool = pcs.enter_context(tc.tile_pool(name="exp", bufs=11))
            nrm_pool = pcs.enter_context(tc.tile_pool(name="nrm", bufs=2))
            s_pool = pcs.enter_context(tc.tile_pool(name="sps", bufs=4, space="PSUM"))
            o_pool = pcs.enter_context(tc.tile_pool(name="ops", bufs=2, space="PSUM"))

            def make_qg(g):
                # Qg^T = Q^T * (Wtalk[g, h] / sqrt(DF)) -- folds the head mix
                qg_sb = qg_pool.tile([128, EC * PH], bf16, tag="qg")
                for ec in range(EC):
                    nc.vector.tensor_scalar_mul(
                        qg_sb[:, ec * PH:(ec + 1) * PH],
                        qt_sb[:, ec * PH:(ec + 1) * PH],
                        wt_sb[:, ec * G + g: ec * G + g + 1])
                return qg_sb

            def normalize(g, o_ps):
                # normalize straight out of PSUM: out^T[df, p] / sums[p]; the
                # ones-row of V' left the denominators in o_ps row DF (the ACT
                # copy also remaps partition DF -> 0 for the reciprocal)
                sum_sb = nrm_pool.tile([1, PH], f32, tag="sum")
                nc.scalar.activation(sum_sb[:], o_ps[DF:DF + 1, :], ACT.Copy)
                r_sb = nrm_pool.tile([1, PH], f32, tag="r")
                nc.vector.reciprocal_approx_fast(r_sb[:], sum_sb[:])
                rb_sb = nrm_pool.tile([DF, PH], f32, tag="rb")
                nc.gpsimd.partition_broadcast(rb_sb[:], r_sb[:])
                po, fo = (g % 2) * DF, (g // 2) * PH
                nc.vector.tensor_mul(
                    ocat_sb[po:po + DF, fo:fo + PH], o_ps[0:DF, :], rb_sb[:])

            # flat software pipeline over all (g, qc) tiles: AV lags QK by LAG
            # tiles ACROSS head boundaries, so the PE stream is uniform and
            # never waits on the exp->bias-multiply chain or head bookkeeping.
            # The V projection is interleaved into the first 8 tiles (V' is
            # first consumed by AV at t=LAG), which keeps it off the serial
            # phase-B critical path.
            NT = G * QC
            LAG = 8
            qg_tiles = {0: make_qg(0)}
            o_pss = {}
            e_all = [None] * NT
            for t in range(NT + LAG):
                if t < NT:
                    g, qc = divmod(t, QC)
                    qg_sb = qg_tiles[g]
                    # b_sb holds exp(bias)^T; softmax as exp(s)*exp(b).
                    # head 0's tiles ride the gpsimd queue so they line up
                    # BEHIND the phase-B loads instead of stealing HBM
                    # bandwidth from them on the concurrent sync queue
                    b_sb = bias_pool.tile([128, PH], bf16, tag="bias")
                    dma_eng = nc.gpsimd if g == 0 else nc.sync
                    dma_eng.dma_start(b_sb[:], biast[g, qc * 128:(qc + 1) * 128, :])
                    e_sb = exp_pool.tile([128, PH], bf16, tag="exp")
                    e_all[t] = e_sb
                    s_pss = []
                    for pc in range(PH // 512):
                        s_ps = s_pool.tile([128, 512], f32, tag="sps")
                        s_pss.append(s_ps)
                        for ec in range(EC):
                            nc.tensor.matmul(
                                s_ps[:],
                                lhsT=kt_sb[:, ec * P + qc * 128: ec * P + (qc + 1) * 128],
                                rhs=qg_sb[:, ec * PH + pc * 512: ec * PH + (pc + 1) * 512],
                                start=(ec == 0), stop=(ec == EC - 1))
                if t < LAG:
                    # V[q, e] natural layout, scattered into V' (2 chunks/tile)
                    for qc_v in (2 * t, 2 * t + 1):
                        v_ps = s_pool.tile([128, 512], f32, tag="sps")
                        for dc in range(DC):
                            nc.tensor.matmul(
                                v_ps[:],
                                lhsT=xt_sb[:, dc * P + qc_v * 128: dc * P + (qc_v + 1) * 128],
                                rhs=wv_sb[:, dc * D:(dc + 1) * D],
                                start=(dc == 0), stop=(dc == DC - 1))
                        dst = v_sb[:, qc_v * G * VW:(qc_v + 1) * G * VW]
                        dst = dst.rearrange("p (g c) -> p g c", c=VW)[:, :, 0:DF]
                        src = v_ps[:].rearrange("p (g c) -> p g c", c=DF)
                        nc.vector.tensor_copy(dst, src)
                if t >= LAG:
                    pg, pqc = divmod(t - LAG, QC)
                    if pqc == 0:
                        op_t = o_pool.tile([VW, PH], f32, tag="ops")
                        o_pss[pg] = op_t
                    for pc in range(PH // 512):
                        nc.tensor.matmul(
                            o_pss[pg][:, pc * 512:(pc + 1) * 512],
                            lhsT=v_sb[:, pqc * G * VW + pg * VW:
                                      pqc * G * VW + (pg + 1) * VW],
                            rhs=e_all[t - LAG][:, pc * 512:(pc + 1) * 512],
                            start=(pqc == 0), stop=(pqc == QC - 1))
                    if pqc == QC - 1:
                        normalize(pg, o_pss.pop(pg))
                if t < NT:
                    for pc in range(PH // 512):
                        nc.scalar.activation(
                            e_sb[:, pc * 512:(pc + 1) * 512], s_pss[pc][:], ACT.Exp)
                        nc.vector.tensor_mul(
                            e_sb[:, pc * 512:(pc + 1) * 512],
                            e_sb[:, pc * 512:(pc + 1) * 512],
                            b_sb[:, pc * 512:(pc + 1) * 512])
                    if qc == 2 and g + 1 < G:
                        # prefetch next head's Qg while DVE is lightly loaded
                        qg_tiles[g + 1] = make_qg(g + 1)

        # ---------- phase D: output projection ----------
        # 4-group waves: each wave runs its ec 0..2 matmuls (heads 0..5, ready
        # early) while the last head's normalize chain finishes; only the ec=3
        # matmuls wait on it. bufs=4 keeps the tiles on the freed s_pool banks,
        # clear of the still-live o_ps banks.
        with ExitStack() as pd:
            y_pool = pd.enter_context(tc.tile_pool(name="yps", bufs=4, space="PSUM"))
            ysb_pool = pd.enter_context(tc.tile_pool(name="ysb", bufs=8))
            for wave in range(2):
                y_pss = []
                for pc4 in range(4):
                    pc = wave * 4 + pc4
                    y_ps = y_pool.tile([128, D], f32, tag="yps")
                    y_pss.append(y_ps)
                    for ec in range(EC - 1):
                        nc.tensor.matmul(
                            y_ps[:],
                            lhsT=ocat_sb[:, ec * PH + pc * 128: ec * PH + (pc + 1) * 128],
                            rhs=wo_sb[:, ec * D:(ec + 1) * D],
                            start=(ec == 0), stop=False)
                for pc4 in range(4):
                    pc = wave * 4 + pc4
                    ec = EC - 1
                    nc.tensor.matmul(
                        y_pss[pc4][:],
                        lhsT=ocat_sb[:, ec * PH + pc * 128: ec * PH + (pc + 1) * 128],
                        rhs=wo_sb[:, ec * D:(ec + 1) * D],
                        start=False, stop=True)
                    y_sb = ysb_pool.tile([128, D], f32, tag="ysb")
                    nc.scalar.activation(y_sb[:], y_pss[pc4][:], ACT.Copy)
                    nc.sync.dma_start(y[pc * 128:(pc + 1) * 128, :], y_sb[:])

    nc.compile()
    return nc


def kernel(x, attn_bias, Wq, Wk, Wv, Wtalk, Wo, **trace_kwargs):
    global LAST_RESULTS
    from concourse.bass_utils import run_bass_kernel_spmd

    x = np.asarray(x, dtype=np.float32)
    attn_bias = np.asarray(attn_bias, dtype=np.float32)
    Wq = np.asarray(Wq, dtype=np.float32)
    Wk = np.asarray(Wk, dtype=np.float32)
    Wv = np.asarray(Wv, dtype=np.float32)
    Wtalk = np.asarray(Wtalk, dtype=np.float32)
    Wo = np.asarray(Wo, dtype=np.float32)

    if "nc" not in _CACHE:
        _CACHE["nc"] = _build_program()
    nc = _CACHE["nc"]

    # host-side layout prep (cheap, reused across cores)
    import ml_dtypes
    bfh = ml_dtypes.bfloat16
    xts = [np.ascontiguousarray(x[b].T).astype(bfh) for b in range(B)]     # [D, P]
    xqts = [[np.ascontiguousarray(x[b, s * PH:(s + 1) * PH, :].T).astype(bfh)
             for s in range(2)] for b in range(B)]                         # [D, PH]
    # ship exp(bias)^T so the kernel can apply bias multiplicatively after exp
    biasts = [np.ascontiguousarray(
        np.exp(attn_bias[0, :, s * PH:(s + 1) * PH, :]).transpose(0, 2, 1))
        .astype(ml_dtypes.bfloat16) for s in range(2)]
    wt = np.ascontiguousarray((np.repeat(Wtalk, DF, axis=1) / np.sqrt(DF)).T
                              .astype(np.float32))                         # [512, 8]

    wq16, wk16 = Wq.astype(bfh), Wk.astype(bfh)
    wv16, wo16 = Wv.astype(bfh), Wo.astype(bfh)
    in_maps = []
    for c in range(N_CORES):
        b, s = c // 2, c % 2
        in_maps.append({
            "xt": xts[b], "xqt": xqts[b][s], "biast": biasts[s],
            "wq": wq16, "wk": wk16, "wv": wv16, "wo": wo16, "wt": wt,
        })

    res = run_bass_kernel_spmd(nc, in_maps, list(range(N_CORES)), **trace_kwargs)
    LAST_RESULTS = res

    out = np.empty((B, P, D), dtype=np.float32)
    for c in range(N_CORES):
        b, s = c // 2, c % 2
        out[b, s * PH:(s + 1) * PH, :] = res.results[c]["y"]
    return out



# revision 47
# speedup vs baseline: 1.0198x; 1.0021x over previous
"""Trainium2 Bass kernel for a talking-heads MHSA block.

Reference computation (B=4, P=2048, D=512, H=8, DF=64, fp32):
    q = (x @ Wq) / sqrt(DF);  k = x @ Wk;  v = x @ Wv      (per-head reshape)
    attn[b,h]   = q_h k_h^T
    attn2[b,g]  = sum_h Wtalk[g,h] attn[b,h]               (talking heads)
    P           = softmax(attn2 + bias, axis=-1)
    out         = concat_g(P_g v_g) @ Wo

Sharding: 8 cores, data-parallel: core c -> batch b=c//2, query-half s=c%2
(1024 query rows, all heads, full 2048 keys). No collectives.

Per-core algorithm (bf16 matmuls, fp32 logits, zero on-chip transposes):
  - host pre-transposes x -> x^T and the bias slice -> bias^T[g, q, p]
  - talking-heads mix is folded into QK: S_mixed[g] = (Wtalk[g,h]/8 * Q)
    contracted over all 512 features against K -> one dense 512-deep matmul
  - S^T[q, p] accumulates in PSUM fp32; DVE adds fp32 bias^T in place; ACT
    computes exp -> bf16 probabilities (no max-subtraction: logits are
    bounded ~+-7, mathematically identical)
  - AV uses exp(S^T) directly as the bf16 moving operand; a ones-column in
    V' produces the softmax denominators in PSUM partition 64 for free
  - normalization is applied after AV (linear), then the output projection
    consumes out^T as lhsT directly.
"""
import sys
from contextlib import ExitStack

import numpy as np

if "/opt/trn_rl_repo" not in sys.path:
    sys.path.insert(0, "/opt/trn_rl_repo")

B, P, D = 4, 2048, 512
H, DF = 8, 64
G = H                 # output head groups
PH = P // 2           # query rows per core
DC = D // 128         # 4 contraction chunks for d
EC = (H * DF) // 128  # 4 chunks for e = (h, df)
QC = P // 128         # 16 key chunks
VW = DF + 1           # V' width per group: 64 cols of V + ones column
N_CORES = 8

_CACHE = {}
LAST_RESULTS = None


def _build_program():
    import concourse.mybir as mybir
    import concourse.tile as tile
    from concourse import bacc

    f32 = mybir.dt.float32
    bf16 = mybir.dt.bfloat16
    ACT = mybir.ActivationFunctionType

    nc = bacc.Bacc("TRN2", target_bir_lowering=False, debug=False)
    # x and the projection weights arrive pre-cast to bf16: halves the HBM
    # read traffic during the DMA-bound staging phase
    xt = nc.dram_tensor("xt", [D, P], bf16, kind="ExternalInput").ap()
    xqt = nc.dram_tensor("xqt", [D, PH], bf16, kind="ExternalInput").ap()
    biast = nc.dram_tensor("biast", [G, P, PH], bf16, kind="ExternalInput").ap()
    wq = nc.dram_tensor("wq", [D, H * DF], bf16, kind="ExternalInput").ap()
    wk = nc.dram_tensor("wk", [D, H * DF], bf16, kind="ExternalInput").ap()
    wv = nc.dram_tensor("wv", [D, H * DF], bf16, kind="ExternalInput").ap()
    wo = nc.dram_tensor("wo", [H * DF, D], bf16, kind="ExternalInput").ap()
    wt = nc.dram_tensor("wt", [H * DF, G], f32, kind="ExternalInput").ap()
    y = nc.dram_tensor("y", [PH, D], bf16, kind="ExternalOutput").ap()

    with tile.TileContext(nc) as tc, ExitStack() as ctx:
        persist = ctx.enter_context(tc.tile_pool(name="persist", bufs=1))
        qt_sb = persist.tile([128, EC * PH], bf16, tag="qt")      # Q^T [e, p]
        kt_sb = persist.tile([128, EC * P], bf16, tag="kt")       # K^T [e, q]
        v_sb = persist.tile([128, QC * G * VW], bf16, tag="v")    # V' [q, g*65+c]
        wo_sb = persist.tile([128, EC * D], bf16, tag="wo")
        wt_sb = persist.tile([128, EC * G], f32, tag="wt")
        ocat_sb = persist.tile([128, EC * PH], bf16, tag="ocat")  # out^T [e, p]

        def cast_load(dst_tile, dram_ap, n):
            # SWDGE cast f32 DRAM -> bf16 SBUF, one DMA per tensor
            nc.gpsimd.dma_start(
                dst_tile[:].rearrange("p (c m) -> p c m", c=n),
                dram_ap.rearrange("(c p) m -> p c m", p=128))

        # staging tiles stay live into phase C (the V projection runs there)
        stage = ctx.enter_context(tc.tile_pool(name="stage", bufs=1))
        xt_sb = stage.tile([128, DC * P], bf16, tag="xt")
        xqt_sb = stage.tile([128, DC * PH], bf16, tag="xqt")
        wq_sb = stage.tile([128, DC * D], bf16, tag="wq")
        wk_sb = stage.tile([128, DC * D], bf16, tag="wk")
        wv_sb = stage.tile([128, DC * D], bf16, tag="wv")

        # ---------- phase B: staging + Q/K projections ----------
        with ExitStack() as pb:
            # DMA order matches the interleaved K/Q emission below: each
            # 512-wide projection block's inputs land just before the PE
            # reaches it, so the PE streams through phase B with one short
            # initial wait instead of idling on whole tensors
            xt_v = xt.rearrange("(c p) m -> p c m", p=128)
            xt_sbv = xt_sb[:].rearrange("p (c m) -> p c m", c=DC)
            xqt_v = xqt.rearrange("(c p) m -> p c m", p=128)
            xqt_sbv = xqt_sb[:].rearrange("p (c m) -> p c m", c=DC)
            cast_load(wk_sb, wk, DC)
            nc.gpsimd.dma_start(xt_sbv[:, :, 0:512], xt_v[:, :, 0:512])
            nc.gpsimd.dma_start(xqt_sbv[:, :, 0:512], xqt_v[:, :, 0:512])
            cast_load(wq_sb, wq, DC)
            nc.gpsimd.dma_start(xt_sbv[:, :, 512:1024], xt_v[:, :, 512:1024])
            nc.gpsimd.dma_start(xqt_sbv[:, :, 512:1024], xqt_v[:, :, 512:1024])
            nc.gpsimd.dma_start(xt_sbv[:, :, 1024:1536], xt_v[:, :, 1024:1536])
            nc.gpsimd.dma_start(xt_sbv[:, :, 1536:2048], xt_v[:, :, 1536:2048])
            cast_load(wv_sb, wv, DC)
            cast_load(wo_sb, wo, EC)
            nc.sync.dma_start(
                wt_sb[:].rearrange("p (c m) -> p c m", c=EC),
                wt.rearrange("(c p) m -> p c m", p=128))

            nc.gpsimd.memset(v_sb[:], 1.0)  # ones columns of V'

            # keep phase B's PSUM footprint at 4 banks so phase C's s_pool can
            # allocate its 4 banks without waiting for all of B's PSUM users
            psA = pb.enter_context(tc.tile_pool(name="psA", bufs=2, space="PSUM"))
            psB = pb.enter_context(tc.tile_pool(name="psB", bufs=2, space="PSUM"))

            def q_block(pc):
                # Q^T[e, p-block] = Wq^T x^T (query half only)
                for ec in range(EC):
                    q_ps = psA.tile([128, 512], f32, tag="qps")
                    for dc in range(DC):
                        nc.tensor.matmul(
                            q_ps[:],
                            lhsT=wq_sb[:, dc * D + ec * 128: dc * D + (ec + 1) * 128],
                            rhs=xqt_sb[:, dc * PH + pc * 512: dc * PH + (pc + 1) * 512],
                            start=(dc == 0), stop=(dc == DC - 1))
                    nc.scalar.activation(
                        qt_sb[:, ec * PH + pc * 512: ec * PH + (pc + 1) * 512],
                        q_ps[:], ACT.Copy)

            def k_block(qn):
                # K^T[e, q-block] over 512 keys
                for ec in range(EC):
                    k_ps = psB.tile([128, 512], f32, tag="kvps")
                    for dc in range(DC):
                        nc.tensor.matmul(
                            k_ps[:],
                            lhsT=wk_sb[:, dc * D + ec * 128: dc * D + (ec + 1) * 128],
                            rhs=xt_sb[:, dc * P + qn * 512: dc * P + (qn + 1) * 512],
                            start=(dc == 0), stop=(dc == DC - 1))
                    nc.scalar.activation(
                        kt_sb[:, ec * P + qn * 512: ec * P + (qn + 1) * 512],
                        k_ps[:], ACT.Copy)

            k_block(0)
            q_block(0)
            k_block(1)
            q_block(1)
            k_block(2)
            k_block(3)

        # ---------- phase C: attention main loop ----------
        with ExitStack() as pcs:
            qg_pool = pcs.enter_context(tc.tile_pool(name="qg", bufs=2))
            bias_pool = pcs.enter_context(tc.tile_pool(name="bias", bufs=8))
            exp_pool = pcs.enter_context(tc.tile_pool(name="exp", bufs=11))
            nrm_pool = pcs.enter_context(tc.tile_pool(name="nrm", bufs=2))
            s_pool = pcs.enter_context(tc.tile_pool(name="sps", bufs=4, space="PSUM"))
            o_pool = pcs.enter_context(tc.tile_pool(name="ops", bufs=2, space="PSUM"))

            def make_qg(g):
                # Qg^T = Q^T * (Wtalk[g, h] / sqrt(DF)) -- folds the head mix
                qg_sb = qg_pool.tile([128, EC * PH], bf16, tag="qg")
                for ec in range(EC):
                    nc.vector.tensor_scalar_mul(
                        qg_sb[:, ec * PH:(ec + 1) * PH],
                        qt_sb[:, ec * PH:(ec + 1) * PH],
                        wt_sb[:, ec * G + g: ec * G + g + 1])
                return qg_sb

            def normalize(g, o_ps):
                # normalize straight out of PSUM: out^T[df, p] / sums[p]; the
                # ones-row of V' left the denominators in o_ps row DF (the ACT
                # copy also remaps partition DF -> 0 for the reciprocal)
                sum_sb = nrm_pool.tile([1, PH], f32, tag="sum")
                nc.scalar.activation(sum_sb[:], o_ps[DF:DF + 1, :], ACT.Copy)
                r_sb = nrm_pool.tile([1, PH], f32, tag="r")
                nc.vector.reciprocal_approx_fast(r_sb[:], sum_sb[:])
                rb_sb = nrm_pool.tile([DF, PH], f32, tag="rb")
                nc.gpsimd.partition_broadcast(rb_sb[:], r_sb[:])
                po, fo = (g % 2) * DF, (g // 2) * PH
                nc.vector.tensor_mul(
                    ocat_sb[po:po + DF, fo:fo + PH], o_ps[0:DF, :], rb_sb[:])

            # flat software pipeline over all (g, qc) tiles: AV lags QK by LAG
            # tiles ACROSS head boundaries, so the PE stream is uniform and
            # never waits on the exp->bias-multiply chain or head bookkeeping.
            # The V projection is interleaved into the first 8 tiles (V' is
            # first consumed by AV at t=LAG), which keeps it off the serial
            # phase-B critical path.
            NT = G * QC
            LAG = 8
            qg_tiles = {0: make_qg(0)}
            o_pss = {}
            e_all = [None] * NT
            for t in range(NT + LAG):
                if t < NT:
                    g, qc = divmod(t, QC)
                    qg_sb = qg_tiles[g]
                    # b_sb holds exp(bias)^T; softmax as exp(s)*exp(b).
                    # head 0's tiles ride the gpsimd queue so they line up
                    # BEHIND the phase-B loads instead of stealing HBM
                    # bandwidth from them on the concurrent sync queue
                    b_sb = bias_pool.tile([128, PH], bf16, tag="bias")
                    dma_eng = nc.gpsimd if g == 0 else nc.sync
                    dma_eng.dma_start(b_sb[:], biast[g, qc * 128:(qc + 1) * 128, :])
                    e_sb = exp_pool.tile([128, PH], bf16, tag="exp")
                    e_all[t] = e_sb
                    s_pss = []
                    for pc in range(PH // 512):
                        s_ps = s_pool.tile([128, 512], f32, tag="sps")
                        s_pss.append(s_ps)
                        for ec in range(EC):
                            nc.tensor.matmul(
                                s_ps[:],
                                lhsT=kt_sb[:, ec * P + qc * 128: ec * P + (qc + 1) * 128],
                                rhs=qg_sb[:, ec * PH + pc * 512: ec * PH + (pc + 1) * 512],
                                start=(ec == 0), stop=(ec == EC - 1))
                if t < LAG:
                    # V[q, e] natural layout, scattered into V' (2 chunks/tile)
                    for qc_v in (2 * t, 2 * t + 1):
                        v_ps = s_pool.tile([128, 512], f32, tag="sps")
                        for dc in range(DC):
                            nc.tensor.matmul(
                                v_ps[:],
                                lhsT=xt_sb[:, dc * P + qc_v * 128: dc * P + (qc_v + 1) * 128],
                                rhs=wv_sb[:, dc * D:(dc + 1) * D],
                                start=(dc == 0), stop=(dc == DC - 1))
                        dst = v_sb[:, qc_v * G * VW:(qc_v + 1) * G * VW]
                        dst = dst.rearrange("p (g c) -> p g c", c=VW)[:, :, 0:DF]
                        src = v_ps[:].rearrange("p (g c) -> p g c", c=DF)
                        nc.vector.tensor_copy(dst, src)
                if t >= LAG:
                    pg, pqc = divmod(t - LAG, QC)
                    if pqc == 0:
                        op_t = o_pool.tile([VW, PH], f32, tag="ops")
                        o_pss[pg] = op_t
                    for pc in range(PH // 512):
                        nc.tensor.matmul(
                            o_pss[pg][:, pc * 512:(pc + 1) * 512],
                            lhsT=v_sb[:, pqc * G * VW + pg * VW:
                                      pqc * G * VW + (pg + 1) * VW],
                            rhs=e_all[t - LAG][:, pc * 512:(pc + 1) * 512],
                            start=(pqc == 0), stop=(pqc == QC - 1))
                    if pqc == QC - 1:
                        normalize(pg, o_pss.pop(pg))
                if t < NT:
                    for pc in range(PH // 512):
                        nc.scalar.activation(
                            e_sb[:, pc * 512:(pc + 1) * 512], s_pss[pc][:], ACT.Exp)
                        nc.vector.tensor_mul(
                            e_sb[:, pc * 512:(pc + 1) * 512],
                            e_sb[:, pc * 512:(pc + 1) * 512],
                            b_sb[:, pc * 512:(pc + 1) * 512])
                    if qc == 2 and g + 1 < G:
                        # prefetch next head's Qg while DVE is lightly loaded
                        qg_tiles[g + 1] = make_qg(g + 1)

        # ---------- phase D: output projection ----------
        # 4-group waves: each wave runs its ec 0..2 matmuls (heads 0..5, ready
        # early) while the last head's normalize chain finishes; only the ec=3
        # matmuls wait on it. bufs=4 keeps the tiles on the freed s_pool banks,
        # clear of the still-live o_ps banks.
        with ExitStack() as pd:
            y_pool = pd.enter_context(tc.tile_pool(name="yps", bufs=4, space="PSUM"))
            ysb_pool = pd.enter_context(tc.tile_pool(name="ysb", bufs=8))
            for wave in range(2):
                y_pss = []
                for pc4 in range(4):
                    pc = wave * 4 + pc4
                    y_ps = y_pool.tile([128, D], f32, tag="yps")
                    y_pss.append(y_ps)
                    for ec in range(EC - 1):
                        nc.tensor.matmul(
                            y_ps[:],
                            lhsT=ocat_sb[:, ec * PH + pc * 128: ec * PH + (pc + 1) * 128],
                            rhs=wo_sb[:, ec * D:(ec + 1) * D],
                            start=(ec == 0), stop=False)
                for pc4 in range(4):
                    pc = wave * 4 + pc4
                    ec = EC - 1
                    nc.tensor.matmul(
                        y_pss[pc4][:],
                        lhsT=ocat_sb[:, ec * PH + pc * 128: ec * PH + (pc + 1) * 128],
                        rhs=wo_sb[:, ec * D:(ec + 1) * D],
                        start=False, stop=True)
                    y_sb = ysb_pool.tile([128, D], bf16, tag="ysb")
                    nc.scalar.activation(y_sb[:], y_pss[pc4][:], ACT.Copy)
                    nc.sync.dma_start(y[pc * 128:(pc + 1) * 128, :], y_sb[:])

    nc.compile()
    return nc


def kernel(x, attn_bias, Wq, Wk, Wv, Wtalk, Wo, **trace_kwargs):
    global LAST_RESULTS
    from concourse.bass_utils import run_bass_kernel_spmd

    x = np.asarray(x, dtype=np.float32)
    attn_bias = np.asarray(attn_bias, dtype=np.float32)
    Wq = np.asarray(Wq, dtype=np.float32)
    Wk = np.asarray(Wk, dtype=np.float32)
    Wv = np.asarray(Wv, dtype=np.float32)
    Wtalk = np.asarray(Wtalk, dtype=np.float32)
    Wo = np.asarray(Wo, dtype=np.float32)

    if "nc" not in _CACHE:
        _CACHE["nc"] = _build_program()
    nc = _CACHE["nc"]

    # host-side layout prep (cheap, reused across cores)
    import ml_dtypes
    bfh = ml_dtypes.bfloat16
    xts = [np.ascontiguousarray(x[b].T).astype(bfh) for b in range(B)]     # [D, P]
    xqts = [[np.ascontiguousarray(x[b, s * PH:(s + 1) * PH, :].T).astype(bfh)
             for s in range(2)] for b in range(B)]                         # [D, PH]
    # ship exp(bias)^T so the kernel can apply bias multiplicatively after exp
    biasts = [np.ascontiguousarray(
        np.exp(attn_bias[0, :, s * PH:(s + 1) * PH, :]).transpose(0, 2, 1))
        .astype(ml_dtypes.bfloat16) for s in range(2)]
    wt = np.ascontiguousarray((np.repeat(Wtalk, DF, axis=1) / np.sqrt(DF)).T
                              .astype(np.float32))                         # [512, 8]

    wq16, wk16 = Wq.astype(bfh), Wk.astype(bfh)
    wv16, wo16 = Wv.astype(bfh), Wo.astype(bfh)
    in_maps = []
    for c in range(N_CORES):
        b, s = c // 2, c % 2
        in_maps.append({
            "xt": xts[b], "xqt": xqts[b][s], "biast": biasts[s],
            "wq": wq16, "wk": wk16, "wv": wv16, "wo": wo16, "wt": wt,
        })

    res = run_bass_kernel_spmd(nc, in_maps, list(range(N_CORES)), **trace_kwargs)
    LAST_RESULTS = res

    out = np.empty((B, P, D), dtype=np.float32)
    for c in range(N_CORES):
        b, s = c // 2, c % 2
        out[b, s * PH:(s + 1) * PH, :] = np.asarray(
            res.results[c]["y"], dtype=np.float32)
    return out

